# revision 24
# baseline (speedup 1.0000x reference)
"""AIG triple embedding layer on 8 TRN2 NeuronCores.

Math: out[t] = W @ concat(src[t], r[t], dst[t]) + b
            = TA[fs[t]] + TB[fd[t]] + (EW0 + b) + rel[t]*(EW1 - EW0)
where the src/dst node tables are DEDUPED: the reference's input and output
sinusoid tables are identical (same n, d), so the flat table is
  U = [sinusoid(256) | gate[:256]]  (512 rows)
  fs = {0,1}->idx, 2->256+idx, 3->512 (out of range -> zero contribution)
  TA = U @ W1.T, TB = U @ W3.T  (host-precomputed weight algebra, fp16)
  EW0+b goes into the ScalarE evacuation bias; (EW1-EW0)*rel is a rank-3
  selector matmul.

Device impl "bselmm" (bucketed row-selection matmuls):
  The host PERMUTES each core's 65536 triples into 16 buckets keyed by
  (fs>>7 clamped to 3, fd>>7 clamped to 3), padding each bucket to a fixed
  capacity of GB groups of 512 triples.  Within one 512-triple PSUM group
  both table lookups then touch a SINGLE 128-row chunk of each table, so the
  device needs only 5 TensorE passes per group instead of 11:
    2 broadcast matmuls (fs_rel / fd_rel to 128 partitions)
    1 TA-chunk one-hot matmul + 1 TB-chunk one-hot matmul + 1 rel matmul
  The host also precomputes fs_rel = fs - 128*ca (so the device compares
  against a single iota column), sends the [3, PT] fp16 row stack
  (fs_rel | fd_rel | rel) directly, and un-permutes the transposed fp16
  output during unsharding.  Type-3 (zero row) triples get fs_rel >= 128
  which never matches the iota -> zero contribution, exactly like the
  un-bucketed kernel.

Sharding: data-parallel over T across 8 cores; tables/weights replicated.
"""

import numpy as np

D = 128
T = 524288
NCORES = 8
NSHARD = T // NCORES  # 65536
NI = 256              # num_input_nodes == num_output_nodes == IDX_MAX
P = 128
GRP = 512             # triples per psum group
NBUCK = 16            # (ca, cb) chunk-pair buckets
GB = 9                # groups per bucket (capacity 4608 >> mean 4096)
NCH = 4               # 128-row chunks in the deduped 512-row table

IMPL = "bselmm"

_CACHE = {}
_DEVPOS = []          # per-core device-column positions of each triple
_POST_CTR = [0]


def _sinusoid(n, d):
    pos = np.arange(n, dtype=np.float32)[:, None]
    div = np.exp(np.arange(0, d, 2, dtype=np.float32)
                 * (-np.log(np.float32(10000.0)) / np.float32(d)))
    ang = (pos * div).astype(np.float32)
    enc = np.zeros((n, d), np.float32)
    enc[:, 0::2] = np.sin(ang)
    enc[:, 1::2] = np.cos(ang)
    return enc


def _build_nc_bselmm(gb_list=None):
    """gb_list: groups per bucket, len NBUCK (static schedule, all cores)."""
    if gb_list is None:
        gb_list = _CACHE.get("last_gb", (GB,) * NBUCK)
    import concourse.bacc as bacc
    import concourse.mybir as mybir
    import concourse.tile as tile
    from concourse import library_config

    f32 = mybir.dt.float32
    f16 = mybir.dt.float16
    AL = mybir.AluOpType
    AF = mybir.ActivationFunctionType

    ngrp = sum(gb_list)
    pt = ngrp * GRP
    # group -> (ca, cb)
    chlist = []
    for b, g in enumerate(gb_list):
        chlist += [(b >> 2, b & 3)] * g

    nc = bacc.Bacc(None, target_bir_lowering=False)

    ta = nc.dram_tensor("ta", [NCH * P, D], f16, kind="ExternalInput")
    tb = nc.dram_tensor("tb", [NCH * P, D], f16, kind="ExternalInput")
    ewdw = nc.dram_tensor("ewdw", [1, D], f16, kind="ExternalInput")
    ew0b = nc.dram_tensor("ew0b", [P, 1], f32, kind="ExternalInput")
    ioc = nc.dram_tensor("ioc", [P, 1], f32, kind="ExternalInput")
    rfs = nc.dram_tensor("rfs", [1, pt], f16, kind="ExternalInput")
    rfd = nc.dram_tensor("rfd", [1, pt], f16, kind="ExternalInput")
    rrl = nc.dram_tensor("rrl", [1, pt], f16, kind="ExternalInput")
    outT = nc.dram_tensor("outT", [P, pt], f16, kind="ExternalOutput")

    GPB = 8                       # groups per sl block
    nblk = (ngrp + GPB - 1) // GPB

    with tile.TileContext(nc) as tc:
        with (
            tc.tile_pool(name="const", bufs=1) as cpool,
            tc.tile_pool(name="psumO", bufs=4, space="PSUM") as pO,
            tc.tile_pool(name="bcast", bufs=4) as xpool,
            tc.tile_pool(name="oh", bufs=4) as ohp,
            tc.tile_pool(name="stage", bufs=2) as spool,
            tc.tile_pool(name="outs", bufs=3) as osp,
        ):
            nc.gpsimd.load_library(library_config.mlp)
            # ---------------- constants ----------------
            TAc = cpool.tile([P, NCH, D], f16)
            nc.sync.dma_start(out=TAc[:],
                              in_=ta[:].rearrange("(c p) d -> p c d", p=P))
            TBc = cpool.tile([P, NCH, D], f16)
            nc.sync.dma_start(out=TBc[:],
                              in_=tb[:].rearrange("(c p) d -> p c d", p=P))
            ioc_sb = cpool.tile([P, 1], f32)
            nc.sync.dma_start(out=ioc_sb[:], in_=ioc[:])
            ew0b_sb = cpool.tile([P, 1], f32)
            nc.sync.dma_start(out=ew0b_sb[:], in_=ew0b[:])
            ewdt = cpool.tile([1, D], f16)
            nc.sync.dma_start(out=ewdt[:], in_=ewdw[:])

            # ---------------- PE warm-up ----------------
            # The PE HAM clock-gate only reaches 2.4 GHz after one fully
            # busy ~3.4us window of full-array (K=128) activity.
            warm = pO.tile([P, 2 * GRP], f32, tag="psO")
            tbflat = TBc[:].rearrange("p c d -> p (c d)")
            for _ in range(12):
                nc.tensor.matmul(out=warm[:, 0:GRP], lhsT=TAc[:, 0, :],
                                 rhs=tbflat[:, 0:GRP], start=True, stop=True)

            # ---------------- main loop ----------------
            for blk in range(nblk):
                g0 = blk * GPB
                gn = min(GPB, ngrp - g0)
                W = gn * GRP
                slf = spool.tile([1, GPB * GRP], f16, tag="slf")
                nc.sync.dma_start(out=slf[:, 0:W],
                                  in_=rfs[:, g0 * GRP:g0 * GRP + W])
                sld = spool.tile([1, GPB * GRP], f16, tag="sld")
                nc.sync.dma_start(out=sld[:, 0:W],
                                  in_=rfd[:, g0 * GRP:g0 * GRP + W])
                slr = spool.tile([1, GPB * GRP], f16, tag="slr")
                nc.sync.dma_start(out=slr[:, 0:W],
                                  in_=rrl[:, g0 * GRP:g0 * GRP + W])

                for gp in range((gn + 1) // 2):
                    # group pair: shared psO / evac / out-DMA
                    nh = 2 if 2 * gp + 1 < gn else 1
                    psO = pO.tile([P, 2 * GRP], f32, tag="psO")
                    for half in range(nh):
                        gi = 2 * gp + half
                        g = g0 + gi
                        ca, cb = chlist[g]
                        cs = slice(gi * GRP, (gi + 1) * GRP)
                        # broadcasts on the (otherwise idle) gpsimd engine
                        FSb = xpool.tile([P, GRP], f16, tag="FSb")
                        nc.gpsimd.partition_broadcast(FSb[:], slf[:, cs])
                        FDb = xpool.tile([P, GRP], f16, tag="FDb")
                        nc.gpsimd.partition_broadcast(FDb[:], sld[:, cs])
                        ohA = ohp.tile([P, GRP], f16, tag="ohA")
                        nc.vector.tensor_scalar(out=ohA[:], in0=FSb[:],
                                                scalar1=ioc_sb[:, 0:1],
                                                scalar2=None,
                                                op0=AL.is_equal)
                        ohB = ohp.tile([P, GRP], f16, tag="ohB")
                        nc.vector.tensor_scalar(out=ohB[:], in0=FDb[:],
                                                scalar1=ioc_sb[:, 0:1],
                                                scalar2=None,
                                                op0=AL.is_equal)
                        po = psO[:, half * GRP:(half + 1) * GRP]
                        nc.tensor.matmul(out=po, lhsT=TAc[:, ca, :],
                                         rhs=ohA[:], start=True, stop=False)
                        nc.tensor.matmul(out=po, lhsT=TBc[:, cb, :],
                                         rhs=ohB[:], start=False, stop=False)
                        nc.tensor.matmul(out=po, lhsT=ewdt[:],
                                         rhs=slr[:, cs], start=False,
                                         stop=True)

                    osb = osp.tile([P, 2 * GRP], f16, tag="osb")
                    nc.scalar.activation(osb[:, 0:nh * GRP],
                                         psO[:, 0:nh * GRP], AF.Identity,
                                         bias=ew0b_sb[:, 0:1])
                    o0 = (g0 + 2 * gp) * GRP
                    nc.sync.dma_start(out=outT[:, o0:o0 + nh * GRP],
                                      in_=osb[:, 0:nh * GRP])

    nc.compile()
    return nc


def _make_in_maps(inputs):
    global _DEVPOS
    gate = np.asarray(inputs["gate_emb"], np.float32)
    edge = np.asarray(inputs["edge_emb"], np.float32)
    W = np.asarray(inputs["W"], np.float32)
    b = np.asarray(inputs["b"], np.float32)

    Utbl = np.concatenate([_sinusoid(NI, D), gate[:NI]], axis=0)  # [512,128]
    W1 = W[:, 0:D]
    W2 = W[:, D:2 * D]
    W3 = W[:, 2 * D:3 * D]
    TA = (Utbl @ W1.T).astype(np.float16)        # [512, 128]
    TB = (Utbl @ W3.T).astype(np.float16)
    ew0b = (edge[0] @ W2.T + b).astype(np.float32).reshape(P, 1)
    ewdw = ((edge[1] - edge[0]) @ W2.T).astype(np.float16).reshape(1, D)
    ioc = np.arange(P, dtype=np.float32).reshape(P, 1)

    idx_names = ["src_idx", "src_type", "rel", "dst_idx", "dst_type"]
    ii = {k: np.asarray(inputs[k]).astype(np.int32) for k in idx_names}

    def flat(idx, typ):
        off = np.maximum(typ - 1, 0) << 8
        return off + np.where(typ == 3, 0, idx)   # in [0, 512]

    fs = flat(ii["src_idx"], ii["src_type"])
    fd = flat(ii["dst_idx"], ii["dst_type"])
    rel = ii["rel"]

    # bucket keys; type-3 (fs==512) clamps into chunk 3 with fs_rel=128
    ka = np.minimum(fs >> 7, 3)
    kb = np.minimum(fd >> 7, 3)
    key = ka * 4 + kb

    # shared static schedule: groups per bucket (same for all cores)
    gb = np.full(NBUCK, GB, np.int64)
    for c in range(NCORES):
        cnt = np.bincount(key[c * NSHARD:(c + 1) * NSHARD], minlength=NBUCK)
        gb = np.maximum(gb, (cnt + GRP - 1) // GRP)
    gb_list = tuple(int(x) for x in gb)
    _CACHE["last_gb"] = gb_list
    ngrp = sum(gb_list)
    pt = ngrp * GRP
    boff = np.concatenate([[0], np.cumsum(gb)[:-1]]) * GRP  # bucket offsets

    common = {"ta": TA, "tb": TB, "ewdw": ewdw, "ew0b": ew0b, "ioc": ioc}

    in_maps = []
    _DEVPOS = []
    _POST_CTR[0] = 0
    for c in range(NCORES):
        sl = slice(c * NSHARD, (c + 1) * NSHARD)
        fsc, fdc, relc, keyc = fs[sl], fd[sl], rel[sl], key[sl]
        order = np.argsort(keyc, kind="stable")
        skey = keyc[order]
        # position within bucket
        within = np.arange(NSHARD) - np.searchsorted(skey, skey, side="left")
        dev_pos_sorted = boff[skey] + within
        dev_pos = np.empty(NSHARD, np.int64)
        dev_pos[order] = dev_pos_sorted

        rfs = np.full((1, pt), 999.0, np.float16)  # padding: never matches
        rfd = np.full((1, pt), 999.0, np.float16)
        rrl = np.zeros((1, pt), np.float16)
        ca_of = np.minimum(fsc >> 7, 3)
        cb_of = np.minimum(fdc >> 7, 3)
        rfs[0, dev_pos] = (fsc - (ca_of << 7)).astype(np.float16)
        rfd[0, dev_pos] = (fdc - (cb_of << 7)).astype(np.float16)
        rrl[0, dev_pos] = relc.astype(np.float16)

        m = dict(common)
        m["rfs"] = rfs
        m["rfd"] = rfd
        m["rrl"] = rrl
        in_maps.append(m)
        _DEVPOS.append(dev_pos)
    return in_maps


def _post(core_result):
    """Device output -> this core's [NSHARD, D] float32 block."""
    c = _POST_CTR[0] % NCORES
    _POST_CTR[0] += 1
    outT = core_result["outT"]
    return outT[:, _DEVPOS[c]].T.astype(np.float32)


BUILDERS = {"bselmm": _build_nc_bselmm}
DEV_OUT = "outT"


def kernel(**inputs):
    from concourse.bass_utils import run_bass_kernel_spmd

    in_maps = _make_in_maps(inputs)
    gb_list = _CACHE["last_gb"]

    if _CACHE.get("gb_list") != gb_list:
        _CACHE["nc"] = _build_nc_bselmm(gb_list)
        _CACHE["gb_list"] = gb_list
    nc = _CACHE["nc"]

    _POST_CTR[0] = 0
    res = run_bass_kernel_spmd(nc, in_maps, core_ids=list(range(NCORES)))
    return np.concatenate([_post(res.results[c]) for c in range(NCORES)],
                          axis=0)
